# revision 31
# baseline (speedup 1.0000x reference)
"""GCN layer kernel for trn2, 8-core SPMD.

out = relu(D^-1/2 (A+I) D^-1/2 (x @ W) + b)
    = relu(dis_d * ((sum_e one_hot(dst_e) dis_s[e] x[src_e]) @ W) + b)

Design (v3):
- dst-shard: core c owns dst nodes [c*12544, (c+1)*12544).
- Edge messages: gather RAW fp32 x rows with the batched `dma_gather`
  custom GPSIMD instruction (mlp ucode library). int16 indices force a
  4-way bank split (src >> 15). Descriptor GENERATION on the Q7 cores is
  the bottleneck (~8.4ns/idx on one queue); 4 SWDGE queues run on 4
  distinct Q7 core pairs, so gathers are split into <=1024-idx chunks
  round-robined over queues (~2.9ns/idx effective).
- Groups of GRP dst tiles pack each bank's edge segments WITHOUT
  per-tile 128-alignment (padding only at bank-run ends); blocks that
  straddle tile boundaries simply get one mask-matmul per tile present.
- Self-loop messages never touch the gather: per dst tile one
  identity-matmul of the dis-scaled own x rows adds dis_d * x_d.
- DVE fuses the fp32->fp16 cast with the dis[src] scale; one-hot masks
  come from is_equal against an iota; scatter-adds are matmuls with
  swapped operands so PSUM accumulates agg^T = (feat x dst), feeding
  the post-aggregation W matmul with no transpose.
- Final dis[dst] scale + relu fused in one tensor_scalar (plus a
  generic 3-op ending when b != 0); fp32 out DMA per tile.
"""
import numpy as np

import concourse.bass as bass
import concourse.mybir as mybir
import concourse.tile as tile
from concourse.bass_utils import run_bass_kernel_spmd
from concourse.library_config import mlp as mlp_lib
from concourse.library_overlay import lower_extended_insts
from concourse.masks import make_identity

# ---- walrus workaround: <=1 sync wait per instruction ----
def _install_tile_patches():
    K = 1

    def _split_waits(tc, ordered):
        nc = tc.nc
        for insts in ordered.values():
            out = []
            for inst in insts:
                si = inst.sync_info
                waits = list(si.on_wait) if si is not None else []
                if len(waits) > K:
                    for i in range(0, len(waits) - K, K):
                        out.append(mybir.InstNoOp(
                            name=nc.get_next_instruction_name(),
                            engine=inst.engine, ins=[], outs=[],
                            sync_info=mybir.SyncInfo(on_wait=waits[i:i+K], on_update=[]),
                        ))
                    inst.sync_info = mybir.SyncInfo(
                        on_wait=waits[len(waits) - K:], on_update=list(si.on_update))
                out.append(inst)
            insts[:] = out

    if getattr(tile.TileContext, "_ant_waitsplit", False):
        return
    orig_lower = tile.TileContext._lower_ordered_insts

    def lower(self, ordered):
        _split_waits(self, ordered)
        return orig_lower(self, ordered)

    def drain(self, tick_clock, wait_clock):
        probe = self.nc.sync.nop(nofuse=True, hint="pre_drain_waits")
        wait_clock.add_sem_waits(probe.ins, tile.ScopedClock({None: tick_clock.global_clock}))
        waits = list(probe.ins.sync_info.on_wait)
        probe.ins.sync_info = mybir.SyncInfo(on_wait=waits[:K], on_update=[])
        for i in range(K, len(waits), K):
            n = self.nc.sync.nop(nofuse=True, hint="pre_drain_waits")
            n.ins.sync_info = mybir.SyncInfo(on_wait=waits[i:i+K], on_update=[])
        self.nc.sync.drain()
        self.nc.all_engine_barrier()
        popped = self.nc._tile_sem_poison_stack.pop()
        assert popped is self._sem_poison
        self.nc.clear_and_free_semaphores(list(self.sems.allocated().values()))
        self.nc.all_engine_barrier()

    tile.TileContext._lower_ordered_insts = lower
    tile.TileContext._drain_and_barrier = drain
    tile.TileContext._ant_waitsplit = True


N = 100000
D = 128
NCORES = 8
P = 128
VPAD = 100352            # 8 * 12544
SH = VPAD // NCORES      # 12544 dst rows per core
NT_SH = SH // P          # 98 dst tiles per core
BANKBITS = 15
BANK = 1 << BANKBITS     # 32768 rows per gather bank (int16 range)
NBANKS = 4
GRP = 4                  # dst tiles per pack/rotation group
CHUNK_BLKS = 7           # max gather chunk: 7 blocks = 896 idxs (57 descs/lane < 64 ring)
NQUEUES = 4

F16 = mybir.dt.float16
F32 = mybir.dt.float32
I16 = mybir.dt.int16


def _host_prep(edge_index):
    """Group-packed, bank-split edge layout + shared block profile."""
    src = edge_index[0].astype(np.int64)
    dst = edge_index[1].astype(np.int64)

    deg = np.bincount(dst, minlength=VPAD).astype(np.float64)
    deg += 1.0                                   # self-loop
    deg[N:] = 1.0
    dis = np.ones(VPAD)
    dis[:N] = 1.0 / np.sqrt(deg[:N])

    NT_ALL = VPAD // P
    order = np.lexsort((src, dst // P))
    src_s = src[order]
    dst_s = dst[order]
    tile_of = dst_s // P
    tile_starts = np.zeros(NT_ALL + 1, dtype=np.int64)
    np.cumsum(np.bincount(tile_of, minlength=NT_ALL), out=tile_starts[1:])

    # per (global tile, bank) edge segments, src-sorted
    seg = {}
    for t in range(NT_ALL):
        a, bnd = tile_starts[t], tile_starts[t + 1]
        es = src_s[a:bnd]
        ed = dst_s[a:bnd]
        bk = es >> BANKBITS
        for b in range(NBANKS):
            m = bk == b
            seg[t, b] = (
                (es[m] - (b << BANKBITS)).astype(np.int16),   # bank-rel idx
                (ed[m] - t * P).astype(np.float16),           # tile-local dst
                dis[es[m]].astype(np.float16),                # dis[src]
            )

    groups = [list(range(g, min(g + GRP, NT_SH))) for g in range(0, NT_SH, GRP)]

    # shared profile: blocks per (group, bank) = ceil(max-core count / 128)
    cnt = np.zeros((NCORES, len(groups), NBANKS), dtype=np.int64)
    for c in range(NCORES):
        for gi, tls in enumerate(groups):
            for b in range(NBANKS):
                cnt[c, gi, b] = sum(len(seg[c * NT_SH + tl, b][0]) for tl in tls)
    NB = np.ceil(cnt.max(axis=0) / P).astype(np.int64)        # [ngroups, NBANKS]

    # block layout: group-major, bank-runs concatenated inside a group
    grp_nblk = NB.sum(axis=1)                                  # blocks per group
    grp_base = np.zeros(len(groups), dtype=np.int64)
    np.cumsum(grp_nblk[:-1], out=grp_base[1:])
    TOTBLK = int(grp_nblk.sum())

    # matmul schedule (shared): per group, per tile: list of
    # (block_pos_in_group, labcol). labcol indexes the label/mask array.
    # A (block, tile) pair exists when ANY core has edges of that tile in
    # that block -- conservative: with the shared profile, per-core edge
    # placement differs, so emit a matmul for every (block, tile) pair
    # where the tile's slot range [lo, hi) intersects the block on ANY
    # core. Using per-core offsets would change the schedule; instead we
    # define slot ranges from per-core cumsum but take the UNION.
    # Simpler and safe: every tile covers the blocks spanned by the union
    # over cores of its [start, end) slot interval within the bank run.
    starts = np.zeros((NCORES, len(groups), NBANKS, GRP + 1), dtype=np.int64)
    for c in range(NCORES):
        for gi, tls in enumerate(groups):
            for b in range(NBANKS):
                off = 0
                for j, tl in enumerate(tls):
                    starts[c, gi, b, j] = off
                    off += len(seg[c * NT_SH + tl, b][0])
                starts[c, gi, b, len(tls):] = off

    mm_sched = []            # [gi][tile_j] -> list of (blockpos, labcol)
    nmat_base = []           # label column base per group
    nmat_total = 0
    for gi, tls in enumerate(groups):
        per_tile = [[] for _ in tls]
        bank_off = 0
        for b in range(NBANKS):
            nb = int(NB[gi, b])
            if nb == 0:
                continue
            for j in range(len(tls)):
                lo = int(starts[:, gi, b, j].min())
                hi = int(starts[:, gi, b, j + 1].max())
                if hi == lo:
                    continue
                blo = lo // P
                bhi = (hi - 1) // P
                for kb in range(blo, bhi + 1):
                    per_tile[j].append((bank_off + kb, None))
            bank_off += nb
        # assign label columns
        for j in range(len(tls)):
            per_tile[j] = [(bp, nmat_total + k)
                           for k, (bp, _) in enumerate(per_tile[j])]
            nmat_total += len(per_tile[j])
        mm_sched.append(per_tile)
        nmat_base.append(nmat_total)
    NMAT = nmat_total

    # per-core arrays. Masks are fully host-built: for matmul column lc
    # of (block bp, tile j), mask[p, d] = dis[src] of the edge at slot
    # (bp, p) if that edge belongs to tile j with local dst d, else 0.
    IDXCOLS = TOTBLK * (P // 16)
    per_core = []
    for c in range(NCORES):
        idx16 = np.zeros((P, IDXCOLS), dtype=np.int16)
        masks = np.zeros((P, NMAT, P), dtype=np.float16)
        for gi, tls in enumerate(groups):
            bank_off = 0
            for b in range(NBANKS):
                nb = int(NB[gi, b])
                if nb == 0:
                    continue
                tot = nb * P
                run_idx = np.zeros(tot, dtype=np.int16)
                off = 0
                lc_of = {}
                for j in range(len(tls)):
                    for bp, lc in mm_sched[gi][j]:
                        rel = bp - bank_off
                        if 0 <= rel < nb:
                            lc_of[j, rel] = lc
                for j, tl in enumerate(tls):
                    si, lb, de = seg[c * NT_SH + tl, b]
                    n = len(si)
                    run_idx[off:off + n] = si
                    slots = np.arange(off, off + n)
                    blk = slots // P
                    prt = slots % P
                    lcs = np.array([lc_of[j, kb] for kb in blk], dtype=np.int64) \
                        if n else np.zeros(0, np.int64)
                    masks[prt, lcs, lb.astype(np.int64)] = de
                    off += n
                if off < tot:
                    run_idx[off:tot] = run_idx[max(0, off - 1)]
                p0 = int(grp_base[gi]) + bank_off
                wrapped = run_idx.reshape(-1, 16).T
                idx16[:, p0 * 8:(p0 + nb) * 8] = np.tile(wrapped, (8, 1))
                bank_off += nb
        per_core.append((idx16, masks.reshape(P, NMAT * P)))

    dis_c = []
    for c in range(NCORES):
        dg = dis[c * SH:(c + 1) * SH]
        dis_c.append(np.ascontiguousarray(
            dg.reshape(NT_SH, P).T.astype(np.float32)))

    meta = dict(groups=groups, NB=NB, grp_base=grp_base, grp_nblk=grp_nblk,
                TOTBLK=TOTBLK, NMAT=NMAT, mm_sched=mm_sched, IDXCOLS=IDXCOLS)
    return per_core, dis_c, meta


def _build_program(meta, bias_zero):
    _install_tile_patches()
    groups = meta["groups"]
    NB = meta["NB"]
    grp_base = meta["grp_base"]
    grp_nblk = meta["grp_nblk"]
    TOTBLK = meta["TOTBLK"]
    NMAT = meta["NMAT"]
    mm_sched = meta["mm_sched"]
    IDXCOLS = meta["IDXCOLS"]

    nc = bass.Bass(num_swdge_queues=NQUEUES)
    x_d = nc.dram_tensor("x", [N, D], F32, kind="ExternalInput")
    W_d = nc.dram_tensor("W", [D, D], F32, kind="ExternalInput")
    b_d = nc.dram_tensor("b", [1, D], F32, kind="ExternalInput")
    idx_d = nc.dram_tensor("idx16", [P, IDXCOLS], I16, kind="ExternalInput")
    mask_d = nc.dram_tensor("masks", [P, NMAT * P], F16, kind="ExternalInput")
    disc_d = nc.dram_tensor("dis_c", [P, NT_SH], F32, kind="ExternalInput")
    out_d = nc.dram_tensor("out", [SH, D], F32, kind="ExternalOutput")

    bank_lo = [b * BANK for b in range(NBANKS)]
    bank_hi = [min((b + 1) * BANK, N) for b in range(NBANKS)]

    with tile.TileContext(nc) as tc:
        nc.gpsimd.load_library(mlp_lib)
        with (
            tc.tile_pool(name="const", bufs=1) as cp,
            tc.tile_pool(name="sg32", bufs=3) as sg32,
            tc.tile_pool(name="sg16", bufs=2) as sg16,
            tc.tile_pool(name="sm", bufs=1) as sm,
            tc.tile_pool(name="sx", bufs=2) as sx,
            tc.tile_pool(name="so", bufs=3) as so,
            tc.tile_pool(name="p1", bufs=4, space="PSUM") as p1,
            tc.tile_pool(name="p2", bufs=2, space="PSUM") as p2,
        ):
            # ---- constants ----
            W_sb = cp.tile([P, P], F32)
            nc.sync.dma_start(out=W_sb[:], in_=W_d[:])
            W16 = cp.tile([P, P], F16)
            nc.vector.tensor_copy(out=W16[:], in_=W_sb[:])
            b_sb = cp.tile([1, P], F32)
            nc.sync.dma_start(out=b_sb[:], in_=b_d[:])
            if not bias_zero:
                b16r = cp.tile([1, P], F16)
                nc.vector.tensor_copy(out=b16r[:], in_=b_sb[:])
                ones16 = cp.tile([1, P], F16)
                nc.vector.memset(ones16[:], 1.0)
                bf_ps = cp.tile([P, P], F32, space="PSUM")
                nc.tensor.matmul(out=bf_ps[:], lhsT=ones16[0:1, :],
                                 rhs=b16r[0:1, :], start=True, stop=True)
                b_full = cp.tile([P, P], F32)
                nc.vector.tensor_copy(out=b_full[:], in_=bf_ps[:])
            ident32 = cp.tile([P, P], F32)
            make_identity(nc, ident32[:])
            ident16 = cp.tile([P, P], F16)
            nc.vector.tensor_copy(out=ident16[:], in_=ident32[:])
            idx_sb = cp.tile([P, IDXCOLS], I16)
            nc.sync.dma_start(out=idx_sb[:], in_=idx_d[:])
            disc_sb = cp.tile([P, NT_SH], F32)
            nc.sync.dma_start(out=disc_sb[:], in_=disc_d[:])

            nidx_regs = {}

            def nidx_reg(v):
                if v not in nidx_regs:
                    nidx_regs[v] = nc.gpsimd.to_reg(v)
                return nidx_regs[v]

            # per-core x shard rows for self-loops: core c needs rows
            # [c*SH, (c+1)*SH) of x. The program is shared; use a
            # per-core input holding just the shard.
            xsh_d = nc.dram_tensor("x_shard", [SH, D], F32, kind="ExternalInput")

            qctr = [0]
            lab_ptr = 0

            for gi, tls in enumerate(groups):
                gbase = int(grp_base[gi])
                nblk_g = int(grp_nblk[gi])
                gbuf32 = sg32.tile([P, nblk_g * P], F32, tag="g32")
                # gathers: per bank run, chunked to <=CHUNK_BLKS blocks
                off = 0
                for b in range(NBANKS):
                    nb = int(NB[gi, b])
                    if nb == 0:
                        continue
                    nchunks = -(-nb // CHUNK_BLKS)
                    csz = [nb // nchunks + (1 if i < nb % nchunks else 0)
                           for i in range(nchunks)]
                    done = 0
                    for nbc in csz:
                        nidx = nbc * P
                        p0 = gbase + off + done
                        nc.gpsimd.dma_gather(
                            out_ap=gbuf32[:, (off + done) * P:(off + done + nbc) * P]
                                .rearrange("p (k q) -> p k q", q=P),
                            in_ap=x_d[bank_lo[b]:bank_hi[b], :],
                            idxs_ap=idx_sb[:, p0 * 8:(p0 + nbc) * 8],
                            num_idxs=nidx,
                            num_idxs_reg=nidx_reg(nidx),
                            elem_size=P,
                            queue_num=qctr[0] % NQUEUES,
                        )
                        qctr[0] += 1
                        done += nbc
                    off += nb

                # plain cast fp32->fp16 on the idle ACT engine (dis[src]
                # lives in the mask values); keeps DVE off the SBUF ports
                # the Q7 SWDGE descriptor rings need.
                gbuf16 = sg16.tile([P, nblk_g * P], F16, tag="g16")
                nc.scalar.copy(out=gbuf16[:], in_=gbuf32[:])
                # host-built dis-weighted one-hot masks for the group
                nmat_g = sum(len(mm_sched[gi][j]) for j in range(len(tls)))
                mgrp = sm.tile([P, nmat_g * P], F16, tag="m")
                nc.sync.dma_start(
                    out=mgrp[:],
                    in_=mask_d[:, lab_ptr * P:(lab_ptr + nmat_g) * P])

                # self-loop x rows for this group's tiles (dis-scaled)
                ntl = len(tls)
                xsl = sx.tile([P, ntl * P], F32, tag="xsl")
                t0 = tls[0]
                nc.sync.dma_start(
                    out=xsl[:].rearrange("p (k q) -> p k q", q=P),
                    in_=xsh_d[t0 * P:(t0 + ntl) * P, :].rearrange(
                        "(k p) q -> p k q", p=P),
                )
                xsl16 = sx.tile([P, ntl * P], F16, tag="xsl16")
                nc.vector.tensor_tensor(
                    out=xsl16[:].rearrange("p (k q) -> p k q", q=P),
                    in0=xsl[:].rearrange("p (k q) -> p k q", q=P),
                    in1=disc_sb[:, t0:t0 + ntl].rearrange(
                        "p (k one) -> p k one", one=1).to_broadcast([P, ntl, P]),
                    op=mybir.AluOpType.mult,
                )

                for j, tl in enumerate(tls):
                    mats = mm_sched[gi][j]
                    ps1 = p1.tile([P, P], F32, space="PSUM", tag="ps1")
                    nmm = len(mats) + 1
                    for k, (bp, lc) in enumerate(mats):
                        nc.tensor.matmul(
                            out=ps1[:],
                            lhsT=gbuf16[:, bp * P:(bp + 1) * P],
                            rhs=mgrp[:, (lc - lab_ptr) * P:(lc - lab_ptr + 1) * P],
                            start=(k == 0), stop=False,
                        )
                    # self-loop: aggT[f, d] += xdis[d, f]
                    nc.tensor.matmul(
                        out=ps1[:], lhsT=xsl16[:, j * P:(j + 1) * P],
                        rhs=ident16[:], start=(len(mats) == 0), stop=True,
                    )
                    aggT16 = so.tile([P, P], F16, tag="aggT")
                    nc.vector.tensor_copy(out=aggT16[:], in_=ps1[:])
                    ps2 = p2.tile([P, P], F32, space="PSUM", tag="ps2")
                    nc.tensor.matmul(
                        out=ps2[:], lhsT=aggT16[:], rhs=W16[:],
                        start=True, stop=True)
                    o = so.tile([P, P], F32, tag="o")
                    if bias_zero:
                        nc.vector.tensor_scalar(
                            out=o[:], in0=ps2[:], scalar1=disc_sb[:, tl:tl + 1],
                            scalar2=0.0, op0=mybir.AluOpType.mult,
                            op1=mybir.AluOpType.max)
                    else:
                        t2 = so.tile([P, P], F32, tag="t2")
                        nc.vector.tensor_scalar(
                            out=t2[:], in0=ps2[:], scalar1=disc_sb[:, tl:tl + 1],
                            scalar2=None, op0=mybir.AluOpType.mult)
                        t3 = so.tile([P, P], F32, tag="t3")
                        nc.vector.tensor_tensor(
                            out=t3[:], in0=t2[:], in1=b_full[:],
                            op=mybir.AluOpType.add)
                        nc.vector.tensor_scalar(
                            out=o[:], in0=t3[:], scalar1=0.0, scalar2=None,
                            op0=mybir.AluOpType.max)
                    nc.scalar.dma_start(out=out_d[tl * P:(tl + 1) * P, :], in_=o[:])
                lab_ptr += nmat_g

    lower_extended_insts(nc)
    return nc


def kernel(x, edge_index, W, b):
    x = np.ascontiguousarray(np.asarray(x, dtype=np.float32))
    edge_index = np.asarray(edge_index)
    W = np.ascontiguousarray(np.asarray(W, dtype=np.float32))
    b = np.asarray(b, dtype=np.float32).reshape(1, D)

    per_core, dis_c, meta = _host_prep(edge_index)

    xpad = np.zeros((VPAD, D), dtype=np.float32)
    xpad[:N] = x

    nc = _build_program(meta, bias_zero=bool(np.all(b == 0)))
    in_maps = []
    for c in range(NCORES):
        idx16, masks = per_core[c]
        in_maps.append({
            "x": x,
            "x_shard": np.ascontiguousarray(xpad[c * SH:(c + 1) * SH]),
            "W": W,
            "b": b,
            "idx16": idx16,
            "masks": masks,
            "dis_c": dis_c[c],
        })
    res = run_bass_kernel_spmd(nc, in_maps, core_ids=list(range(NCORES)), trace=False)
    out = np.concatenate([res.results[c]["out"] for c in range(NCORES)], axis=0)
    return out[:N]


# revision 33
# speedup vs baseline: 1.1461x; 1.1461x over previous
"""GCN layer kernel for trn2, 8-core SPMD.

out = relu(D^-1/2 (A+I) D^-1/2 (x @ W) + b)
    = relu(dis_d * ((sum_e one_hot(dst_e) dis_s[e] x[src_e]) @ W) + b)

Design (v3):
- dst-shard: core c owns dst nodes [c*12544, (c+1)*12544).
- Edge messages: gather RAW fp32 x rows with the batched `dma_gather`
  custom GPSIMD instruction (mlp ucode library). int16 indices force a
  4-way bank split (src >> 15). Descriptor GENERATION on the Q7 cores is
  the bottleneck (~8.4ns/idx on one queue); 4 SWDGE queues run on 4
  distinct Q7 core pairs, so gathers are split into <=1024-idx chunks
  round-robined over queues (~2.9ns/idx effective).
- Groups of GRP dst tiles pack each bank's edge segments WITHOUT
  per-tile 128-alignment (padding only at bank-run ends); blocks that
  straddle tile boundaries simply get one mask-matmul per tile present.
- Self-loop messages never touch the gather: per dst tile one
  identity-matmul of the dis-scaled own x rows adds dis_d * x_d.
- One-hot scatter masks are HOST-built (pure index data) with the
  dis[src] scale folded into the mask values, shipped as a DRAM input
  and DMA'd per group: cheaper than DVE is_equal broadcasts, and keeps
  DVE off the SBUF ports the Q7 descriptor rings contend on.
- The fp32->fp16 cast of gathered rows runs on the idle ACT engine.
- Scatter-adds are matmuls with swapped operands so PSUM accumulates
  agg^T = (feat x dst), feeding the post-aggregation W matmul with no
  transpose. Final dis[dst] scale + relu fused in one tensor_scalar
  (generic 3-op ending when b != 0); fp32 out DMA per tile.
"""
import numpy as np

import concourse.bass as bass
import concourse.mybir as mybir
import concourse.tile as tile
from concourse.bass_utils import run_bass_kernel_spmd
from concourse.library_config import mlp as mlp_lib
from concourse.library_overlay import lower_extended_insts
from concourse.masks import make_identity

# ---- walrus workaround: <=1 sync wait per instruction ----
def _install_tile_patches():
    K = 1

    def _split_waits(tc, ordered):
        nc = tc.nc
        for insts in ordered.values():
            out = []
            for inst in insts:
                si = inst.sync_info
                waits = list(si.on_wait) if si is not None else []
                if len(waits) > K:
                    for i in range(0, len(waits) - K, K):
                        out.append(mybir.InstNoOp(
                            name=nc.get_next_instruction_name(),
                            engine=inst.engine, ins=[], outs=[],
                            sync_info=mybir.SyncInfo(on_wait=waits[i:i+K], on_update=[]),
                        ))
                    inst.sync_info = mybir.SyncInfo(
                        on_wait=waits[len(waits) - K:], on_update=list(si.on_update))
                out.append(inst)
            insts[:] = out

    if getattr(tile.TileContext, "_ant_waitsplit", False):
        return
    orig_lower = tile.TileContext._lower_ordered_insts

    def lower(self, ordered):
        _split_waits(self, ordered)
        return orig_lower(self, ordered)

    def drain(self, tick_clock, wait_clock):
        probe = self.nc.sync.nop(nofuse=True, hint="pre_drain_waits")
        wait_clock.add_sem_waits(probe.ins, tile.ScopedClock({None: tick_clock.global_clock}))
        waits = list(probe.ins.sync_info.on_wait)
        probe.ins.sync_info = mybir.SyncInfo(on_wait=waits[:K], on_update=[])
        for i in range(K, len(waits), K):
            n = self.nc.sync.nop(nofuse=True, hint="pre_drain_waits")
            n.ins.sync_info = mybir.SyncInfo(on_wait=waits[i:i+K], on_update=[])
        self.nc.sync.drain()
        self.nc.all_engine_barrier()
        popped = self.nc._tile_sem_poison_stack.pop()
        assert popped is self._sem_poison
        self.nc.clear_and_free_semaphores(list(self.sems.allocated().values()))
        self.nc.all_engine_barrier()

    tile.TileContext._lower_ordered_insts = lower
    tile.TileContext._drain_and_barrier = drain
    tile.TileContext._ant_waitsplit = True


N = 100000
D = 128
NCORES = 8
P = 128
VPAD = 100352            # 8 * 12544
SH = VPAD // NCORES      # 12544 dst rows per core
NT_SH = SH // P          # 98 dst tiles per core
BANKBITS = 15
BANK = 1 << BANKBITS     # 32768 rows per gather bank (int16 range)
NBANKS = 4
GRP = 3                  # dst tiles per pack/rotation group
CHUNK_BLKS = 7           # max gather chunk: 7 blocks = 896 idxs (57 descs/lane < 64 ring)
NQUEUES = 4

F16 = mybir.dt.float16
F32 = mybir.dt.float32
I16 = mybir.dt.int16


def _host_prep(edge_index):
    """Group-packed, bank-split edge layout + shared block profile."""
    src = edge_index[0].astype(np.int64)
    dst = edge_index[1].astype(np.int64)

    deg = np.bincount(dst, minlength=VPAD).astype(np.float64)
    deg += 1.0                                   # self-loop
    deg[N:] = 1.0
    dis = np.ones(VPAD)
    dis[:N] = 1.0 / np.sqrt(deg[:N])

    NT_ALL = VPAD // P
    order = np.lexsort((src, dst // P))
    src_s = src[order]
    dst_s = dst[order]
    tile_of = dst_s // P
    tile_starts = np.zeros(NT_ALL + 1, dtype=np.int64)
    np.cumsum(np.bincount(tile_of, minlength=NT_ALL), out=tile_starts[1:])

    # per (global tile, bank) edge segments, src-sorted
    seg = {}
    for t in range(NT_ALL):
        a, bnd = tile_starts[t], tile_starts[t + 1]
        es = src_s[a:bnd]
        ed = dst_s[a:bnd]
        bk = es >> BANKBITS
        for b in range(NBANKS):
            m = bk == b
            seg[t, b] = (
                (es[m] - (b << BANKBITS)).astype(np.int16),   # bank-rel idx
                (ed[m] - t * P).astype(np.float16),           # tile-local dst
                dis[es[m]].astype(np.float16),                # dis[src]
            )

    groups = [list(range(g, min(g + GRP, NT_SH))) for g in range(0, NT_SH, GRP)]

    # shared profile: blocks per (group, bank) = ceil(max-core count / 128)
    cnt = np.zeros((NCORES, len(groups), NBANKS), dtype=np.int64)
    for c in range(NCORES):
        for gi, tls in enumerate(groups):
            for b in range(NBANKS):
                cnt[c, gi, b] = sum(len(seg[c * NT_SH + tl, b][0]) for tl in tls)
    NB = np.ceil(cnt.max(axis=0) / P).astype(np.int64)        # [ngroups, NBANKS]

    # block layout: group-major, bank-runs concatenated inside a group
    grp_nblk = NB.sum(axis=1)                                  # blocks per group
    grp_base = np.zeros(len(groups), dtype=np.int64)
    np.cumsum(grp_nblk[:-1], out=grp_base[1:])
    TOTBLK = int(grp_nblk.sum())

    # matmul schedule (shared): per group, per tile: list of
    # (block_pos_in_group, labcol). labcol indexes the label/mask array.
    # A (block, tile) pair exists when ANY core has edges of that tile in
    # that block -- conservative: with the shared profile, per-core edge
    # placement differs, so emit a matmul for every (block, tile) pair
    # where the tile's slot range [lo, hi) intersects the block on ANY
    # core. Using per-core offsets would change the schedule; instead we
    # define slot ranges from per-core cumsum but take the UNION.
    # Simpler and safe: every tile covers the blocks spanned by the union
    # over cores of its [start, end) slot interval within the bank run.
    starts = np.zeros((NCORES, len(groups), NBANKS, GRP + 1), dtype=np.int64)
    for c in range(NCORES):
        for gi, tls in enumerate(groups):
            for b in range(NBANKS):
                off = 0
                for j, tl in enumerate(tls):
                    starts[c, gi, b, j] = off
                    off += len(seg[c * NT_SH + tl, b][0])
                starts[c, gi, b, len(tls):] = off

    mm_sched = []            # [gi][tile_j] -> list of (blockpos, labcol)
    nmat_base = []           # label column base per group
    nmat_total = 0
    for gi, tls in enumerate(groups):
        per_tile = [[] for _ in tls]
        bank_off = 0
        for b in range(NBANKS):
            nb = int(NB[gi, b])
            if nb == 0:
                continue
            for j in range(len(tls)):
                lo = int(starts[:, gi, b, j].min())
                hi = int(starts[:, gi, b, j + 1].max())
                if hi == lo:
                    continue
                blo = lo // P
                bhi = (hi - 1) // P
                for kb in range(blo, bhi + 1):
                    per_tile[j].append((bank_off + kb, None))
            bank_off += nb
        # assign label columns
        for j in range(len(tls)):
            per_tile[j] = [(bp, nmat_total + k)
                           for k, (bp, _) in enumerate(per_tile[j])]
            nmat_total += len(per_tile[j])
        mm_sched.append(per_tile)
        nmat_base.append(nmat_total)
    NMAT = nmat_total

    # per-core arrays. Masks are fully host-built: for matmul column lc
    # of (block bp, tile j), mask[p, d] = dis[src] of the edge at slot
    # (bp, p) if that edge belongs to tile j with local dst d, else 0.
    IDXCOLS = TOTBLK * (P // 16)
    per_core = []
    for c in range(NCORES):
        idx16 = np.zeros((P, IDXCOLS), dtype=np.int16)
        masks = np.zeros((P, NMAT, P), dtype=np.float16)
        for gi, tls in enumerate(groups):
            bank_off = 0
            for b in range(NBANKS):
                nb = int(NB[gi, b])
                if nb == 0:
                    continue
                tot = nb * P
                run_idx = np.zeros(tot, dtype=np.int16)
                off = 0
                lc_of = {}
                for j in range(len(tls)):
                    for bp, lc in mm_sched[gi][j]:
                        rel = bp - bank_off
                        if 0 <= rel < nb:
                            lc_of[j, rel] = lc
                for j, tl in enumerate(tls):
                    si, lb, de = seg[c * NT_SH + tl, b]
                    n = len(si)
                    run_idx[off:off + n] = si
                    slots = np.arange(off, off + n)
                    blk = slots // P
                    prt = slots % P
                    lcs = np.array([lc_of[j, kb] for kb in blk], dtype=np.int64) \
                        if n else np.zeros(0, np.int64)
                    masks[prt, lcs, lb.astype(np.int64)] = de
                    off += n
                if off < tot:
                    run_idx[off:tot] = run_idx[max(0, off - 1)]
                p0 = int(grp_base[gi]) + bank_off
                wrapped = run_idx.reshape(-1, 16).T
                idx16[:, p0 * 8:(p0 + nb) * 8] = np.tile(wrapped, (8, 1))
                bank_off += nb
        per_core.append((idx16, masks.reshape(P, NMAT * P)))

    dis_c = []
    for c in range(NCORES):
        dg = dis[c * SH:(c + 1) * SH]
        dis_c.append(np.ascontiguousarray(
            dg.reshape(NT_SH, P).T.astype(np.float32)))

    meta = dict(groups=groups, NB=NB, grp_base=grp_base, grp_nblk=grp_nblk,
                TOTBLK=TOTBLK, NMAT=NMAT, mm_sched=mm_sched, IDXCOLS=IDXCOLS)
    return per_core, dis_c, meta


def _build_program(meta, bias_zero):
    _install_tile_patches()
    groups = meta["groups"]
    NB = meta["NB"]
    grp_base = meta["grp_base"]
    grp_nblk = meta["grp_nblk"]
    TOTBLK = meta["TOTBLK"]
    NMAT = meta["NMAT"]
    mm_sched = meta["mm_sched"]
    IDXCOLS = meta["IDXCOLS"]

    nc = bass.Bass(num_swdge_queues=NQUEUES)
    x_d = nc.dram_tensor("x", [N, D], F32, kind="ExternalInput")
    W_d = nc.dram_tensor("W", [D, D], F32, kind="ExternalInput")
    b_d = nc.dram_tensor("b", [1, D], F32, kind="ExternalInput")
    idx_d = nc.dram_tensor("idx16", [P, IDXCOLS], I16, kind="ExternalInput")
    mask_d = nc.dram_tensor("masks", [P, NMAT * P], F16, kind="ExternalInput")
    disc_d = nc.dram_tensor("dis_c", [P, NT_SH], F32, kind="ExternalInput")
    out_d = nc.dram_tensor("out", [SH, D], F32, kind="ExternalOutput")

    bank_lo = [b * BANK for b in range(NBANKS)]
    bank_hi = [min((b + 1) * BANK, N) for b in range(NBANKS)]

    with tile.TileContext(nc) as tc:
        nc.gpsimd.load_library(mlp_lib)
        with (
            tc.tile_pool(name="const", bufs=1) as cp,
            tc.tile_pool(name="sg32", bufs=4) as sg32,
            tc.tile_pool(name="sg16", bufs=2) as sg16,
            tc.tile_pool(name="sm", bufs=2) as sm,
            tc.tile_pool(name="sx", bufs=2) as sx,
            tc.tile_pool(name="so", bufs=3) as so,
            tc.tile_pool(name="p1", bufs=4, space="PSUM") as p1,
            tc.tile_pool(name="p2", bufs=2, space="PSUM") as p2,
        ):
            # ---- constants ----
            W_sb = cp.tile([P, P], F32)
            nc.sync.dma_start(out=W_sb[:], in_=W_d[:])
            W16 = cp.tile([P, P], F16)
            nc.vector.tensor_copy(out=W16[:], in_=W_sb[:])
            b_sb = cp.tile([1, P], F32)
            nc.sync.dma_start(out=b_sb[:], in_=b_d[:])
            if not bias_zero:
                b16r = cp.tile([1, P], F16)
                nc.vector.tensor_copy(out=b16r[:], in_=b_sb[:])
                ones16 = cp.tile([1, P], F16)
                nc.vector.memset(ones16[:], 1.0)
                bf_ps = cp.tile([P, P], F32, space="PSUM")
                nc.tensor.matmul(out=bf_ps[:], lhsT=ones16[0:1, :],
                                 rhs=b16r[0:1, :], start=True, stop=True)
                b_full = cp.tile([P, P], F32)
                nc.vector.tensor_copy(out=b_full[:], in_=bf_ps[:])
            ident32 = cp.tile([P, P], F32)
            make_identity(nc, ident32[:])
            ident16 = cp.tile([P, P], F16)
            nc.vector.tensor_copy(out=ident16[:], in_=ident32[:])
            idx_sb = cp.tile([P, IDXCOLS], I16)
            nc.sync.dma_start(out=idx_sb[:], in_=idx_d[:])
            disc_sb = cp.tile([P, NT_SH], F32)
            nc.sync.dma_start(out=disc_sb[:], in_=disc_d[:])

            nidx_regs = {}

            def nidx_reg(v):
                if v not in nidx_regs:
                    nidx_regs[v] = nc.gpsimd.to_reg(v)
                return nidx_regs[v]

            # per-core x shard rows for self-loops: core c needs rows
            # [c*SH, (c+1)*SH) of x. The program is shared; use a
            # per-core input holding just the shard.
            xsh_d = nc.dram_tensor("x_shard", [SH, D], F32, kind="ExternalInput")

            qctr = [0]
            lab_ptr = 0

            for gi, tls in enumerate(groups):
                gbase = int(grp_base[gi])
                nblk_g = int(grp_nblk[gi])
                gbuf32 = sg32.tile([P, nblk_g * P], F32, tag="g32")
                # gathers: per bank run, chunked to <=CHUNK_BLKS blocks
                off = 0
                for b in range(NBANKS):
                    nb = int(NB[gi, b])
                    if nb == 0:
                        continue
                    nchunks = -(-nb // CHUNK_BLKS)
                    csz = [nb // nchunks + (1 if i < nb % nchunks else 0)
                           for i in range(nchunks)]
                    done = 0
                    for nbc in csz:
                        nidx = nbc * P
                        p0 = gbase + off + done
                        nc.gpsimd.dma_gather(
                            out_ap=gbuf32[:, (off + done) * P:(off + done + nbc) * P]
                                .rearrange("p (k q) -> p k q", q=P),
                            in_ap=x_d[bank_lo[b]:bank_hi[b], :],
                            idxs_ap=idx_sb[:, p0 * 8:(p0 + nbc) * 8],
                            num_idxs=nidx,
                            num_idxs_reg=nidx_reg(nidx),
                            elem_size=P,
                            queue_num=qctr[0] % NQUEUES,
                        )
                        qctr[0] += 1
                        done += nbc
                    off += nb

                # plain cast fp32->fp16 on the idle ACT engine (dis[src]
                # lives in the mask values); keeps DVE off the SBUF ports
                # the Q7 SWDGE descriptor rings need.
                gbuf16 = sg16.tile([P, nblk_g * P], F16, tag="g16")
                nc.scalar.copy(out=gbuf16[:], in_=gbuf32[:])
                # host-built dis-weighted one-hot masks for the group
                nmat_g = sum(len(mm_sched[gi][j]) for j in range(len(tls)))
                mgrp = sm.tile([P, nmat_g * P], F16, tag="m")
                nc.sync.dma_start(
                    out=mgrp[:],
                    in_=mask_d[:, lab_ptr * P:(lab_ptr + nmat_g) * P])

                # self-loop x rows for this group's tiles (dis-scaled)
                ntl = len(tls)
                xsl = sx.tile([P, ntl * P], F32, tag="xsl")
                t0 = tls[0]
                nc.sync.dma_start(
                    out=xsl[:].rearrange("p (k q) -> p k q", q=P),
                    in_=xsh_d[t0 * P:(t0 + ntl) * P, :].rearrange(
                        "(k p) q -> p k q", p=P),
                )
                xsl16 = sx.tile([P, ntl * P], F16, tag="xsl16")
                nc.vector.tensor_tensor(
                    out=xsl16[:].rearrange("p (k q) -> p k q", q=P),
                    in0=xsl[:].rearrange("p (k q) -> p k q", q=P),
                    in1=disc_sb[:, t0:t0 + ntl].rearrange(
                        "p (k one) -> p k one", one=1).to_broadcast([P, ntl, P]),
                    op=mybir.AluOpType.mult,
                )

                for j, tl in enumerate(tls):
                    mats = mm_sched[gi][j]
                    ps1 = p1.tile([P, P], F32, space="PSUM", tag="ps1")
                    nmm = len(mats) + 1
                    for k, (bp, lc) in enumerate(mats):
                        nc.tensor.matmul(
                            out=ps1[:],
                            lhsT=gbuf16[:, bp * P:(bp + 1) * P],
                            rhs=mgrp[:, (lc - lab_ptr) * P:(lc - lab_ptr + 1) * P],
                            start=(k == 0), stop=False,
                        )
                    # self-loop: aggT[f, d] += xdis[d, f]
                    nc.tensor.matmul(
                        out=ps1[:], lhsT=xsl16[:, j * P:(j + 1) * P],
                        rhs=ident16[:], start=(len(mats) == 0), stop=True,
                    )
                    aggT16 = so.tile([P, P], F16, tag="aggT")
                    nc.vector.tensor_copy(out=aggT16[:], in_=ps1[:])
                    ps2 = p2.tile([P, P], F32, space="PSUM", tag="ps2")
                    nc.tensor.matmul(
                        out=ps2[:], lhsT=aggT16[:], rhs=W16[:],
                        start=True, stop=True)
                    o = so.tile([P, P], F32, tag="o")
                    if bias_zero:
                        nc.vector.tensor_scalar(
                            out=o[:], in0=ps2[:], scalar1=disc_sb[:, tl:tl + 1],
                            scalar2=0.0, op0=mybir.AluOpType.mult,
                            op1=mybir.AluOpType.max)
                    else:
                        t2 = so.tile([P, P], F32, tag="t2")
                        nc.vector.tensor_scalar(
                            out=t2[:], in0=ps2[:], scalar1=disc_sb[:, tl:tl + 1],
                            scalar2=None, op0=mybir.AluOpType.mult)
                        t3 = so.tile([P, P], F32, tag="t3")
                        nc.vector.tensor_tensor(
                            out=t3[:], in0=t2[:], in1=b_full[:],
                            op=mybir.AluOpType.add)
                        nc.vector.tensor_scalar(
                            out=o[:], in0=t3[:], scalar1=0.0, scalar2=None,
                            op0=mybir.AluOpType.max)
                    nc.scalar.dma_start(out=out_d[tl * P:(tl + 1) * P, :], in_=o[:])
                lab_ptr += nmat_g

    lower_extended_insts(nc)
    return nc


def kernel(x, edge_index, W, b):
    x = np.ascontiguousarray(np.asarray(x, dtype=np.float32))
    edge_index = np.asarray(edge_index)
    W = np.ascontiguousarray(np.asarray(W, dtype=np.float32))
    b = np.asarray(b, dtype=np.float32).reshape(1, D)

    per_core, dis_c, meta = _host_prep(edge_index)

    xpad = np.zeros((VPAD, D), dtype=np.float32)
    xpad[:N] = x

    nc = _build_program(meta, bias_zero=bool(np.all(b == 0)))
    in_maps = []
    for c in range(NCORES):
        idx16, masks = per_core[c]
        in_maps.append({
            "x": x,
            "x_shard": np.ascontiguousarray(xpad[c * SH:(c + 1) * SH]),
            "W": W,
            "b": b,
            "idx16": idx16,
            "masks": masks,
            "dis_c": dis_c[c],
        })
    res = run_bass_kernel_spmd(nc, in_maps, core_ids=list(range(NCORES)), trace=False)
    out = np.concatenate([res.results[c]["out"] for c in range(NCORES)], axis=0)
    return out[:N]
